# revision 23
# baseline (speedup 1.0000x reference)
"""TP-8 Trainium2 Bass kernel for a LLaDA/Llama transformer block (v9).

Shapes (hardcoded): x [2, 1024, 4096], 32 heads x 128 head_dim,
FF=12288, non-causal attention, RMSNorm + RoPE + SwiGLU.

Sharding: tensor-parallel over 8 cores - q/k/v/ff sharded on the
output-feature axis (4 heads / 1536 ff dims per core), wo/w_out sharded
on the contraction axis.  One fp16 AllReduce per batch restores the
residual stream; final projection partials are summed on the host.

Structure:
 - Software-pipelined across the two batches: batch 0's AllReduce and
   norm2/MLP overlap batch 1's attention/o-proj and vice versa.
 - fp8e4 DoubleRow matmuls (half-rate rows) for q/k/v/o projections,
   softmax denominator, PV, and rms-norm square-sums.  Weights carry
   power-of-two host scales (SWQ..SWO); Sa/Sv activation scales fold
   into PSUM evictions.  Logits and the MLP stay fp16.
 - rms_norm(1) folds into the projection evictions (cc*rs1 / ss*rs1 for
   q,k via rope; a per-token [P,1] scale for v built with 1-col ones
   matmuls), so projections consume raw fp8 x.
 - The AllReduce output is read once per batch into a resident
   [128, 32, T] fp16 tile reused by norm2 stats, ff/up matmuls and the
   wout residual re-read path.
 - The gpsimd queue carries ONLY collectives (a collective trigger
   blocks its queue until completion).  norm2 square passes carry an
   artificial anchor dependency (stt with a bypass scalar operand) so
   the scheduler cannot hoist them ahead of latency-critical DVE work;
   their PSUM reduction matmuls are interleaved into the next MLP
   phase via hooks.
"""

from contextlib import ExitStack

import numpy as np
import ml_dtypes

import concourse.mybir as mybir
import concourse.tile as tile
from concourse import bacc
from concourse.bass_utils import run_bass_kernel_spmd

F32 = mybir.dt.float32
F16 = mybir.dt.float16
F8 = mybir.dt.float8e4
AF = mybir.ActivationFunctionType
ALU = mybir.AluOpType
DR = mybir.MatmulPerfMode.DoubleRow
E4 = ml_dtypes.float8_e4m3

N_CORES = 8
P = 128
B, T, D, FF = 2, 1024, 4096, 12288
M = B * T
H = 128
HALF = 64
QC = D // N_CORES
NH = QC // H
FC = FF // N_CORES
NKP = D // P
NPR = NKP // 2
NFT = FC // P
NDT = D // P
NST = T // P
EPS = 1e-05

SWQ = 512.0
SWK = 64.0
SWV = 64.0
SWO = 32.0
SA = 16.0
SV = 4.0


def _build():
    nc = bacc.Bacc("TRN2", target_bir_lowering=False, num_devices=N_CORES)

    x8d = nc.declare_dram_parameter("x8d", [D, M], F8, isOutput=False)
    xh8 = nc.declare_dram_parameter("xh8", [D, M], F16, isOutput=False)  # x/8
    css = nc.declare_dram_parameter("css", [2, P, M], F16, isOutput=False)
    wq8 = nc.declare_dram_parameter("wq8", [NH, P, NKP, P], F8, isOutput=False)
    wk8 = nc.declare_dram_parameter("wk8", [NH, P, NKP, P], F8, isOutput=False)
    wv8 = nc.declare_dram_parameter("wv8", [P, NKP, QC], F8, isOutput=False)
    wo8 = nc.declare_dram_parameter("wo8", [P, NH, NDT, P], F8, isOutput=False)
    wf_t = nc.declare_dram_parameter("wf_t", [NFT, P, NKP, P], F16, isOutput=False)
    wu_t = nc.declare_dram_parameter("wu_t", [NFT, P, NKP, P], F16, isOutput=False)
    wout_t = nc.declare_dram_parameter("wout_t", [NDT, P, NFT, P], F16, isOutput=False)
    y = nc.declare_dram_parameter("y", [D, M], F32, isOutput=True)

    with tile.TileContext(nc) as tc:
        _emit(nc, tc, x8d, xh8, css, wq8, wk8, wv8, wo8, wf_t, wu_t, wout_t, y)
    nc.compile()
    return nc


def _emit(nc, tc, x8d, xh8, css, wq8, wk8, wv8, wo8, wf_t, wu_t, wout_t, y):
    top = ExitStack()
    with top:
        dram_pool = top.enter_context(tc.tile_pool(name="dram", bufs=1, space="DRAM"))
        const = top.enter_context(tc.tile_pool(name="const", bufs=1))

        cc_in = [dram_pool.tile([D, T], F16, name=f"cc_in_{b}") for b in range(B)]
        cc_out = [
            dram_pool.tile([D, T], F16, addr_space="Shared", name=f"cc_out_{b}")
            for b in range(B)
        ]

        ones8 = const.tile([P, 2, P], F8)
        nc.vector.memset(ones8[:], 1.0)
        onescol = const.tile([P, 1], F16)
        nc.vector.memset(onescol[:], SV / (SWV * P))
        eps_sb = const.tile([P, 1], F32)
        nc.vector.memset(eps_sb[:], EPS)
        lnav = const.tile([P, 1], F32)
        nc.vector.memset(lnav[:], float(np.log(SA / SV)))
        bc2 = [const.tile([P, T], F16, name=f"bc2_{b}") for b in range(B)]

        # shared stats pools: one ms psum tile (rotated over the 4 norm
        # passes) + a small ring of fp8 square-pair tiles
        stsb = top.enter_context(tc.tile_pool(name="stsb", bufs=1))
        stps = top.enter_context(tc.tile_pool(name="stps", bufs=1, space="PSUM"))

        # xm pool: one tag-rotated buffer; xm[1]'s DMA WAR-waits on the
        # last ff/up read of xm[0].
        xmp = top.enter_context(tc.tile_pool(name="xmp", bufs=1))

        # ---- first-half state; batch-alternating tiles share tags ----
        half1 = ExitStack()
        ep = half1.enter_context(tc.tile_pool(name="half1", bufs=1))
        cc_sb = [ep.tile([P, T], F16, tag="ccs", name=f"ccs_{b}") for b in range(B)]
        ss_sb = [ep.tile([P, T], F16, tag="sss", name=f"sss_{b}") for b in range(B)]

        def emit_css_dma(b):
            bs = slice(b * T, (b + 1) * T)
            nc.sync.dma_start(out=cc_sb[b][:], in_=css[0][:, bs])
            nc.sync.dma_start(out=ss_sb[b][:], in_=css[1][:, bs])

        bc1 = [ep.tile([P, T], F16, tag="bc1", name=f"bc1_{b}") for b in range(B)]
        ccrs = [ep.tile([P, T], F16, tag="ccrs", name=f"ccrs_{b}") for b in range(B)]
        ssrs = [ep.tile([P, T], F16, tag="ssrs", name=f"ssrs_{b}") for b in range(B)]
        pt_sb = [ep.tile([P, NST], F32, tag="pt", name=f"pt_{b}") for b in range(B)]
        x8t = [ep.tile([P, NKP, T], F8, tag="x8", name=f"x8_{b}") for b in range(B)]

        def w_qk_dma(b, which):
            wsrc = wq8 if which == "q" else wk8
            out = []
            for m in range(NH):
                t = ep.tile([P, NKP, P], F8, tag="wqk", bufs=3,
                            name=f"w{which}_{b}_{m}")
                nc.sync.dma_start(out=t[:], in_=wsrc[m])
                out.append(t)
            return out

        def w_v_dma(b):
            t = ep.tile([P, NKP, QC], F8, tag="wv", bufs=1, name=f"wv_{b}")
            nc.sync.dma_start(out=t[:], in_=wv8[:])
            return t

        def w_o_dma(b):
            t = ep.tile([P, NH, NDT, P], F8, tag="wo", bufs=1, name=f"wo_{b}")
            nc.sync.dma_start(out=t[:], in_=wo8[:])
            return t

        def emit_x8_dma(b):
            bs = slice(b * T, (b + 1) * T)
            for kp in range(NKP):
                nc.sync.dma_start(
                    out=x8t[b][:, kp, :], in_=x8d[kp * P : (kp + 1) * P, bs]
                )

        # ---------------- stats helpers ----------------
        def new_ms(tag):
            return stps.tile([P, T], F32, tag="ms", name=f"ms_{tag}")

        def stats_sq_inline(src3d, ms_ps, tag, eng="mixed"):
            # squares + reduction matmuls emitted together (norm1 passes)
            for pi in range(NPR):
                sq = stsb.tile([P, 2, T], F8, tag="sq", bufs=3,
                               name=f"sq_{tag}_{pi}")
                for j in (0, 1):
                    kp = 2 * pi + j
                    if eng == "dve" or kp % 2 == 1:
                        nc.vector.tensor_mul(
                            sq[:, j, :], src3d[:, kp, :], src3d[:, kp, :]
                        )
                    else:
                        nc.scalar.activation(
                            sq[:, j, :], src3d[:, kp, :], AF.Square
                        )
                for ch in range(T // 512):
                    cs = slice(ch * 512, (ch + 1) * 512)
                    nc.tensor.matmul(
                        ms_ps[:, cs],
                        ones8[:],
                        sq[:, :, cs],
                        start=(pi == 0),
                        stop=(pi == NPR - 1),
                        perf_mode=DR,
                    )

        def stats_sq_anchored(src3d, tag, anchor):
            # DVE squares with an artificial dependency on `anchor` (a
            # [P,1] slice of a tile written by the op the squares must
            # not be scheduled ahead of); reduction mms come later.
            sqs = []
            for pi in range(NPR):
                sq = stsb.tile([P, 2, T], F8, tag="sq", bufs=3,
                               name=f"sq_{tag}_{pi}")
                for j in (0, 1):
                    kp = 2 * pi + j
                    nc.vector.scalar_tensor_tensor(
                        sq[:, j, :], src3d[:, kp, :], anchor,
                        src3d[:, kp, :], ALU.bypass, ALU.mult,
                    )
                sqs.append(sq)
            return sqs

        def stats_mms(sqs, ms_ps, rng):
            for pi in rng:
                for ch in range(T // 512):
                    cs = slice(ch * 512, (ch + 1) * 512)
                    nc.tensor.matmul(
                        ms_ps[:, cs],
                        ones8[:],
                        sqs[pi][:, :, cs],
                        start=(pi == 0),
                        stop=(pi == NPR - 1),
                        perf_mode=DR,
                    )

        def stats_finish(ms_ps, bcast_out, tag):
            lnt = stsb.tile([P, T], F16, tag="lnt", name=f"lnt_{tag}")
            nc.scalar.activation(
                lnt[:], ms_ps[:], AF.Ln, bias=eps_sb[:], scale=1.0 / D
            )
            nc.scalar.activation(bcast_out[:], lnt[:], AF.Exp, scale=-0.5)

        def emit_pt(b, ms_ps):
            # pt[p, st] = rs1[st*P+p] * SV/SWV via 1-col ones matmuls
            for st in range(NST):
                nc.tensor.matmul(
                    ms_ps[:, st : st + 1],
                    bc1[b][:, st * P : (st + 1) * P],
                    onescol[:],
                    start=True,
                    stop=True,
                )
            nc.scalar.activation(pt_sb[b][:], ms_ps[:, 0:NST], AF.Copy)

        def emit_ccss(b):
            nc.vector.tensor_mul(ccrs[b][:], cc_sb[b][:], bc1[b][:])
            nc.vector.tensor_mul(ssrs[b][:], ss_sb[b][:], bc1[b][:])

        # ---------------- phase helpers ----------------
        def emit_qk(b, sp, ap, qpp, qf, kf, insert_pt, ms_ps, wqt, wkt):
            for which, wts, s_w, dst in (
                ("q", wqt, 1.0 / SWQ, qf),
                ("k", wkt, 1.0 / SWK, kf),
            ):
                for m in range(NH):
                    wt = wts[m]
                    ps = qpp.tile([P, T], F32, tag="qk", bufs=2,
                                  name=f"ps{which}_{b}_{m}")
                    for pi in range(NPR):
                        for ch in range(T // 512):
                            cs = slice(ch * 512, (ch + 1) * 512)
                            nc.tensor.matmul(
                                ps[:, cs],
                                wt[:, 2 * pi : 2 * pi + 2, :],
                                x8t[b][:, 2 * pi : 2 * pi + 2, cs],
                                start=(pi == 0),
                                stop=(pi == NPR - 1),
                                perf_mode=DR,
                            )
                    if insert_pt and which == "q" and m == 0:
                        emit_pt(b, ms_ps)
                    main = sp.tile([P, T], F16, tag="rmain", bufs=2,
                                   name=f"rm_{which}_{b}_{m}")
                    nc.vector.scalar_tensor_tensor(
                        main[:], ps[:], s_w, ccrs[b][:], ALU.mult, ALU.mult
                    )
                    rot = sp.tile([P, T], F16, tag="rrot", bufs=2,
                                  name=f"rr_{which}_{b}_{m}")
                    nc.vector.scalar_tensor_tensor(
                        rot[:HALF], ps[HALF:], -s_w, ssrs[b][:HALF],
                        ALU.mult, ALU.mult,
                    )
                    nc.vector.scalar_tensor_tensor(
                        rot[HALF:], ps[:HALF], s_w, ssrs[b][HALF:],
                        ALU.mult, ALU.mult,
                    )
                    out = ap.tile([P, T], F16, tag=f"{which}f{m}",
                                  name=f"{which}f_{b}_{m}")
                    nc.vector.tensor_add(out[:], main[:], rot[:])
                    dst[m] = out

        def emit_v(b, vpp, v8_sb, wv_sb):
            for st in range(NST):
                ps = vpp.tile([P, QC], F32, tag="vps", bufs=2,
                              name=f"psv_{b}_{st}")
                for pi in range(NPR):
                    nc.tensor.matmul(
                        ps[:],
                        x8t[b][:, 2 * pi : 2 * pi + 2, st * P : (st + 1) * P],
                        wv_sb[:, 2 * pi : 2 * pi + 2, :],
                        start=(pi == 0),
                        stop=(pi == NPR - 1),
                        perf_mode=DR,
                    )
                nc.scalar.activation(
                    v8_sb[:, st, :], ps[:], AF.Copy,
                    scale=pt_sb[b][:, st : st + 1],
                )

        def emit_attn_head(b, h, ap_, app, qf, kf, v8_sb, af8b):
            den_ps = app.tile([P, T], F32, tag="den", name=f"den_{b}_{h}")
            at_ps = app.tile([P, T], F32, tag="at", name=f"at_{b}_{h}")
            prs = [None] * (NST // 2)

            def emit_lg(u):
                pr8 = ap_.tile([P, 2, T], F8, tag="pr8", bufs=3,
                               name=f"pr_{b}_{h}_{u}")
                for j in (0, 1):
                    st = 2 * u + j
                    for ch in range(T // 512):
                        cs = slice(ch * 512, (ch + 1) * 512)
                        lg_ps = app.tile([P, 512], F32, tag="lg", bufs=2,
                                         name=f"lg_{b}_{h}_{st}_{ch}")
                        nc.tensor.matmul(
                            lg_ps[:],
                            kf[h][:, st * P : (st + 1) * P],
                            qf[h][:, cs],
                            start=True,
                            stop=True,
                        )
                        nc.scalar.activation(pr8[:, j, cs], lg_ps[:], AF.Exp)
                prs[u] = pr8

            emit_lg(0)
            for u in range(NST // 2):
                if u + 1 < NST // 2:
                    emit_lg(u + 1)
                pr8 = prs[u]
                for ch in range(T // 512):
                    cs = slice(ch * 512, (ch + 1) * 512)
                    nc.tensor.matmul(
                        den_ps[:, cs],
                        ones8[:],
                        pr8[:, :, cs],
                        start=(u == 0),
                        stop=(u == NST // 2 - 1),
                        perf_mode=DR,
                    )
                for ch in range(T // 512):
                    cs = slice(ch * 512, (ch + 1) * 512)
                    nc.tensor.matmul(
                        at_ps[:, cs],
                        v8_sb[:, 2 * u : 2 * u + 2, h * H : (h + 1) * H],
                        pr8[:, :, cs],
                        start=(u == 0),
                        stop=(u == NST // 2 - 1),
                        perf_mode=DR,
                    )
            # af = at * (SA/SV) / den  via  exp(ln(SA/SV) - ln(den))
            lnd = ap_.tile([P, T], F16, tag="lnd", bufs=2, name=f"lnd_{b}_{h}")
            nc.scalar.activation(lnd[:], den_ps[:], AF.Ln)
            rs = ap_.tile([P, T], F16, tag="rsd", bufs=2, name=f"rsd_{b}_{h}")
            nc.scalar.activation(
                rs[:], lnd[:], AF.Exp, scale=-1.0, bias=lnav[:]
            )
            nc.vector.tensor_mul(af8b[:, h, :], at_ps[:], rs[:])

        def emit_oproj(b, sp, opp, af8b, wo_sb):
            bs = slice(b * T, (b + 1) * T)

            def xh_dma(dt):
                t = sp.tile([P, T], F16, tag="xh", bufs=3, name=f"xh_{b}_{dt}")
                nc.sync.dma_start(
                    out=t[:], in_=xh8[dt * P : (dt + 1) * P, bs]
                )
                return t
            xhl = [xh_dma(dt) for dt in range(2)] + [None] * (NDT - 2)
            for dt in range(NDT):
                if dt + 2 < NDT:
                    xhl[dt + 2] = xh_dma(dt + 2)
                ps = opp.tile([P, T], F32, tag="ops", bufs=3,
                              name=f"pso_{b}_{dt}")
                for u in range(NH // 2):
                    for ch in range(T // 512):
                        cs = slice(ch * 512, (ch + 1) * 512)
                        nc.tensor.matmul(
                            ps[:, cs],
                            wo_sb[:, 2 * u : 2 * u + 2, dt, :],
                            af8b[:, 2 * u : 2 * u + 2, cs],
                            start=(u == 0),
                            stop=(u == NH // 2 - 1),
                            perf_mode=DR,
                        )
                osb = sp.tile([P, T], F16, tag="osb", bufs=2,
                              name=f"osb_{b}_{dt}")
                nc.vector.scalar_tensor_tensor(
                    osb[:], ps[:], 1.0 / (SA * SWO), xhl[dt][:],
                    ALU.mult, ALU.add,
                )
                nc.sync.dma_start(
                    out=cc_in[b][dt * P : (dt + 1) * P, :], in_=osb[:]
                )
            nc.gpsimd.collective_compute(
                "AllReduce",
                ALU.add,
                replica_groups=[list(range(N_CORES))],
                ins=[cc_in[b][:, :]],
                outs=[cc_out[b][:, :]],
            )

        xm = [None, None]

        def emit_xm_dma(b):
            xm[b] = xmp.tile([P, NKP, T], F16, tag="xm", bufs=1, name=f"xm_{b}")
            for kp in range(NKP):
                nc.sync.dma_start(
                    out=xm[b][:, kp, :],
                    in_=cc_out[b][kp * P : (kp + 1) * P, :],
                )

        def emit_mlp(b, sp, hp, stat_hook=None):
            # When stat_hook is given, bc2[b] is only produced inside the
            # hook at (m=1, 'f'); evictions of earlier blocks must be
            # deferred until then (they read bc2).  PSUM bufs=3 holds the
            # deferred accumulators.
            hsb = []
            ffs_tiles = {}

            def evict(m, which, ps):
                nt = sp.tile([P, T], F16, tag=f"nt_{which}", bufs=2,
                             name=f"nt{which}_{b}_{m}")
                nc.vector.scalar_tensor_tensor(
                    nt[:], ps[:], 1.0, bc2[b][:], ALU.mult, ALU.mult
                )
                if which == "f":
                    ffs = sp.tile([P, T], F16, tag="ffs", bufs=2,
                                  name=f"ff_{b}_{m}")
                    nc.scalar.activation(ffs[:], nt[:], AF.Silu)
                    ffs_tiles[m] = ffs
                else:
                    ht = hp.tile([P, T], F16, tag=f"h{m}", name=f"h_{b}_{m}")
                    nc.vector.tensor_mul(ht[:], nt[:], ffs_tiles[m][:])
                    hsb.append(ht)

            bc_ready = stat_hook is None
            deferred = []
            with ExitStack() as psc:
                pp = psc.enter_context(
                    tc.tile_pool(name=f"mlpp{b}", bufs=1, space="PSUM")
                )
                for m in range(NFT):
                    for which, wsrc in (("f", wf_t), ("u", wu_t)):
                        wt = sp.tile([P, NKP, P], F16, tag="wffu", bufs=2,
                                     name=f"w{which}_{b}_{m}")
                        nc.sync.dma_start(out=wt[:], in_=wsrc[m])
                        ps = pp.tile([P, T], F32, tag="psfu", bufs=3,
                                     name=f"ps{which}_{b}_{m}")
                        for kp in range(NKP):
                            for ch in range(T // 512):
                                cs = slice(ch * 512, (ch + 1) * 512)
                                nc.tensor.matmul(
                                    ps[:, cs],
                                    wt[:, kp, :],
                                    xm[b][:, kp, cs],
                                    start=(kp == 0),
                                    stop=(kp == NKP - 1),
                                )
                        if stat_hook is not None:
                            if stat_hook(m, which):
                                bc_ready = True
                                for dm, dw, dps in deferred:
                                    evict(dm, dw, dps)
                                deferred = []
                        if bc_ready:
                            evict(m, which, ps)
                        else:
                            deferred.append((m, which, ps))
            return hsb

        def emit_wout(b, sp, pp, hsb):
            bs = slice(b * T, (b + 1) * T)
            for dt in range(NDT):
                wt = sp.tile([P, NFT, P], F16, tag="wot", bufs=3,
                             name=f"wot_{b}_{dt}")
                nc.sync.dma_start(out=wt[:], in_=wout_t[dt])
                ps = pp.tile([P, T], F32, tag="pso2", bufs=2,
                             name=f"pso2_{b}_{dt}")
                for m in range(NFT):
                    for ch in range(T // 512):
                        cs = slice(ch * 512, (ch + 1) * 512)
                        nc.tensor.matmul(
                            ps[:, cs],
                            wt[:, m, :],
                            hsb[m][:, cs],
                            start=(m == 0),
                            stop=(m == NFT - 1),
                        )
                xr = sp.tile([P, T], F16, tag="xr", bufs=3,
                             name=f"xr_{b}_{dt}")
                nc.sync.dma_start(
                    out=xr[:],
                    in_=cc_out[b][dt * P : (dt + 1) * P, :],
                )
                ysb = sp.tile([P, T], F32, tag="ysb", bufs=3,
                              name=f"ysb_{b}_{dt}")
                nc.vector.scalar_tensor_tensor(
                    ysb[:], xr[:], 0.125, ps[:], ALU.mult, ALU.add
                )
                nc.sync.dma_start(out=y[dt * P : (dt + 1) * P, bs], in_=ysb[:])

        def make_stat_hook(sqs, ms_ps, bc_out, tag):
            def hook(m, which):
                if m == 0 and which == "f":
                    stats_mms(sqs, ms_ps, range(0, 6))
                elif m == 0 and which == "u":
                    stats_mms(sqs, ms_ps, range(6, 12))
                elif m == 1 and which == "f":
                    stats_mms(sqs, ms_ps, range(12, NPR))
                    stats_finish(ms_ps, bc_out, tag)
                    return True
                return False
            return hook

        # ================= emission schedule =================
        emit_x8_dma(0)
        emit_css_dma(0)
        wqt0 = w_qk_dma(0, "q")
        wkt0 = w_qk_dma(0, "k")
        wvt0 = w_v_dma(0)

        qf = [[None] * NH for _ in range(B)]
        kf = [[None] * NH for _ in range(B)]
        v8_sb = [ep.tile([P, NST, QC], F8, tag="v8", name=f"v8_{b}")
                 for b in range(B)]
        af8 = [ep.tile([P, NH, T], F8, tag="af8", name=f"af8_{b}")
               for b in range(B)]

        # ---- b0 projections ----
        with ExitStack() as ph:
            sp = ph.enter_context(tc.tile_pool(name="prj0", bufs=1))
            qpp = ph.enter_context(tc.tile_pool(name="qk0", bufs=1, space="PSUM"))
            vpp = ph.enter_context(tc.tile_pool(name="v0", bufs=1, space="PSUM"))
            ms_n1b0 = new_ms("n1b0")
            stats_sq_inline(x8t[0], ms_n1b0, "n1b0")
            stats_finish(ms_n1b0, bc1[0], "n1b0")
            emit_ccss(0)
            emit_qk(0, sp, ep, qpp, qf[0], kf[0], True, ms_n1b0, wqt0, wkt0)
            emit_v(0, vpp, v8_sb[0], wvt0)

        # ---- b0 attention (+ b1 x8 / wq / wo prefetch) ----
        with ExitStack() as ah:
            ap_ = ah.enter_context(tc.tile_pool(name="attb0", bufs=1))
            app = ah.enter_context(tc.tile_pool(name="attp0", bufs=1, space="PSUM"))
            emit_attn_head(0, 0, ap_, app, qf[0], kf[0], v8_sb[0], af8[0])
            emit_x8_dma(1)
            emit_css_dma(1)
            emit_attn_head(0, 1, ap_, app, qf[0], kf[0], v8_sb[0], af8[0])
            wqt1 = w_qk_dma(1, "q")
            wo0 = w_o_dma(0)
            for h in range(2, NH):
                emit_attn_head(0, h, ap_, app, qf[0], kf[0], v8_sb[0], af8[0])

        # ---- b1 norm1 stats (DVE squares) ----
        ms_n1b1 = new_ms("n1b1")
        stats_sq_inline(x8t[1], ms_n1b1, "n1b1", eng="dve")
        stats_finish(ms_n1b1, bc1[1], "n1b1")
        emit_ccss(1)
        emit_pt(1, ms_n1b1)

        # ---- b1 q/k, then b0 o-proj + AR0, then b1 v ----
        with ExitStack() as ph:
            sp = ph.enter_context(tc.tile_pool(name="prj1", bufs=1))
            wkt1 = w_qk_dma(1, "k")
            wvt1 = w_v_dma(1)
            with ExitStack() as qh:
                qpp = qh.enter_context(
                    tc.tile_pool(name="qk1", bufs=1, space="PSUM")
                )
                emit_qk(1, sp, ep, qpp, qf[1], kf[1], False, None, wqt1, wkt1)

            with ExitStack() as oh:
                osp = oh.enter_context(tc.tile_pool(name="op0", bufs=1))
                opp = oh.enter_context(
                    tc.tile_pool(name="opp0", bufs=1, space="PSUM")
                )
                emit_oproj(0, osp, opp, af8[0], wo0)

            with ExitStack() as vh:
                vpp = vh.enter_context(tc.tile_pool(name="v1", bufs=1, space="PSUM"))
                emit_v(1, vpp, v8_sb[1], wvt1)

        # ---- b1 attention (+ b0 xm prefetch) ----
        with ExitStack() as ah:
            ap_ = ah.enter_context(tc.tile_pool(name="attb1", bufs=1))
            app = ah.enter_context(tc.tile_pool(name="attp1", bufs=1, space="PSUM"))
            emit_attn_head(1, 0, ap_, app, qf[1], kf[1], v8_sb[1], af8[1])
            wo1 = w_o_dma(1)
            emit_xm_dma(0)
            for h in range(1, NH):
                emit_attn_head(1, h, ap_, app, qf[1], kf[1], v8_sb[1], af8[1])

        # ---- b1 o-proj + AR1 ----
        with ExitStack() as oh:
            osp = oh.enter_context(tc.tile_pool(name="op1", bufs=1))
            opp = oh.enter_context(tc.tile_pool(name="opp1", bufs=1, space="PSUM"))
            emit_oproj(1, osp, opp, af8[1], wo1)

        # ---- b0 norm2 squares: anchored behind b1's last attention
        # eviction so they fill DVE gaps during b1's o-proj ----
        sq0 = stats_sq_anchored(xm[0], "n2b0", af8[1][:, NH - 1, 0:1])
        ms_n2b0 = new_ms("n2b0")

        half1.close()

        # ---- b0 MLP (stats mms for b0 norm2 interleaved via hook) ----
        with ExitStack() as mh:
            sp = mh.enter_context(tc.tile_pool(name="mlp0", bufs=1))
            hp = mh.enter_context(tc.tile_pool(name="h0", bufs=1))
            hsb = emit_mlp(
                0, sp, hp,
                stat_hook=make_stat_hook(sq0, ms_n2b0, bc2[0], "n2b0"),
            )
            emit_xm_dma(1)
            with ExitStack() as wh:
                wsp = wh.enter_context(tc.tile_pool(name="wo2_0", bufs=1))
                wpp = wh.enter_context(
                    tc.tile_pool(name="wo2p0", bufs=1, space="PSUM")
                )
                emit_wout(0, wsp, wpp, hsb)

            # b1 norm2 squares: anchored behind b0's last MLP h-mul;
            # they fill DVE gaps during b0's wout phase
            sq1 = stats_sq_anchored(xm[1], "n2b1", hsb[NFT - 1][:, 0:1])
            ms_n2b1 = new_ms("n2b1")

        # ---- b1 MLP (+ b1 norm2 stats mms via hook) ----
        with ExitStack() as mh1:
            sp = mh1.enter_context(tc.tile_pool(name="mlp1", bufs=1))
            hp = mh1.enter_context(tc.tile_pool(name="h1", bufs=1))
            hsb = emit_mlp(
                1, sp, hp,
                stat_hook=make_stat_hook(sq1, ms_n2b1, bc2[1], "n2b1"),
            )
            with ExitStack() as wh:
                wsp = wh.enter_context(tc.tile_pool(name="wo2_1", bufs=1))
                wpp = wh.enter_context(
                    tc.tile_pool(name="wo2p1", bufs=1, space="PSUM")
                )
                emit_wout(1, wsp, wpp, hsb)


_NC_CACHE = {}


def _get_nc():
    if "nc" not in _NC_CACHE:
        _NC_CACHE["nc"] = _build()
    return _NC_CACHE["nc"]


def _host_prep(x, sin, cos, attn_norm_w, ff_norm_w, wq, wk, wv, wo, w_ff, w_up, w_out):
    f16 = np.float16
    x2 = np.asarray(x, np.float32).reshape(M, D)
    xT = np.ascontiguousarray(x2.T)

    sinT = np.asarray(sin, np.float32).reshape(M, HALF).T
    cosT = np.asarray(cos, np.float32).reshape(M, HALF).T
    cc = np.concatenate([cosT, cosT], axis=0)
    ss = np.concatenate([sinT, sinT], axis=0)
    css = np.stack([cc, ss]).astype(f16)

    anw = np.asarray(attn_norm_w, np.float32)[:, None]
    fnw = np.asarray(ff_norm_w, np.float32)[:, None]
    wqn = (anw * np.asarray(wq, np.float32)) * (H ** -0.5) * SWQ
    wkn = anw * np.asarray(wk, np.float32) * SWK
    wvn = anw * np.asarray(wv, np.float32) * SWV
    won = np.asarray(wo, np.float32) * SWO
    wfn = fnw * np.asarray(w_ff, np.float32)
    wun = fnw * np.asarray(w_up, np.float32)
    w_out = np.asarray(w_out, np.float32)

    def mtile(w):
        # [K, F] -> [F/P, P, K/P, P] with [m, p, kp, j] = w[kp*P+p, m*P+j]
        K, F = w.shape
        return np.ascontiguousarray(
            w.reshape(K // P, P, F // P, P).transpose(2, 1, 0, 3)
        )

    x8_full = xT.astype(E4)
    xh8_full = (xT * 0.125).astype(f16)

    in_maps = []
    for c in range(N_CORES):
        qs = slice(c * QC, (c + 1) * QC)
        fs = slice(c * FC, (c + 1) * FC)
        in_maps.append(
            {
                "x8d": x8_full,
                "xh8": xh8_full,
                "css": css,
                "wq8": mtile(wqn[:, qs]).astype(E4),
                "wk8": mtile(wkn[:, qs]).astype(E4),
                # [p, kp, f] = wvn[kp*P+p, f]
                "wv8": np.ascontiguousarray(
                    wvn[:, qs].reshape(NKP, P, QC).transpose(1, 0, 2)
                ).astype(E4),
                # [p, h, dt, j] = wo[c*QC + h*P + p, dt*P + j]
                "wo8": np.ascontiguousarray(
                    won[qs, :].reshape(NH, P, NDT, P).transpose(1, 0, 2, 3)
                ).astype(E4),
                "wf_t": mtile(wfn[:, fs]).astype(f16),
                "wu_t": mtile(wun[:, fs]).astype(f16),
                "wout_t": mtile(w_out[fs, :]).astype(f16),
            }
        )
    return in_maps


def kernel(**inputs) -> np.ndarray:
    nc = _get_nc()
    in_maps = _host_prep(**inputs)
    res = run_bass_kernel_spmd(
        nc, in_maps, core_ids=list(range(N_CORES)), trace=False
    )
    acc = res.results[0]["y"].astype(np.float64)
    for c in range(1, N_CORES):
        acc += res.results[c]["y"]
    return np.ascontiguousarray(acc.T).astype(np.float32).reshape(B, T, D)


# revision 24
# speedup vs baseline: 1.0434x; 1.0434x over previous
"""TP-8 Trainium2 Bass kernel for a LLaDA/Llama transformer block (v9).

Shapes (hardcoded): x [2, 1024, 4096], 32 heads x 128 head_dim,
FF=12288, non-causal attention, RMSNorm + RoPE + SwiGLU.

Sharding: tensor-parallel over 8 cores - q/k/v/ff sharded on the
output-feature axis (4 heads / 1536 ff dims per core), wo/w_out sharded
on the contraction axis.  One fp16 AllReduce per batch restores the
residual stream; final projection partials are summed on the host.

Structure:
 - Software-pipelined across the two batches: batch 0's AllReduce and
   norm2/MLP overlap batch 1's attention/o-proj and vice versa.
 - fp8e4 DoubleRow matmuls (half-rate rows) for q/k/v/o projections,
   softmax denominator, PV, and rms-norm square-sums.  Weights carry
   power-of-two host scales (SWQ..SWO); Sa/Sv activation scales fold
   into PSUM evictions.  Logits and the MLP stay fp16.
 - rms_norm(1) folds into the projection evictions (cc*rs1 / ss*rs1 for
   q,k via rope; a per-token [P,1] scale for v built with 1-col ones
   matmuls), so projections consume raw fp8 x.
 - The AllReduce output is read once per batch into a resident
   [128, 32, T] fp16 tile reused by norm2 stats, ff/up matmuls and the
   wout residual re-read path.
 - The gpsimd queue carries ONLY collectives (a collective trigger
   blocks its queue until completion).  norm2 square passes carry an
   artificial anchor dependency (stt with a bypass scalar operand) so
   the scheduler cannot hoist them ahead of latency-critical DVE work;
   their PSUM reduction matmuls are interleaved into the next MLP
   phase via hooks.
"""

from contextlib import ExitStack

import numpy as np
import ml_dtypes

import concourse.mybir as mybir
import concourse.tile as tile
from concourse import bacc
from concourse.bass_utils import run_bass_kernel_spmd

F32 = mybir.dt.float32
F16 = mybir.dt.float16
F8 = mybir.dt.float8e4
AF = mybir.ActivationFunctionType
ALU = mybir.AluOpType
DR = mybir.MatmulPerfMode.DoubleRow
E4 = ml_dtypes.float8_e4m3

N_CORES = 8
P = 128
B, T, D, FF = 2, 1024, 4096, 12288
M = B * T
H = 128
HALF = 64
QC = D // N_CORES
NH = QC // H
FC = FF // N_CORES
NKP = D // P
NPR = NKP // 2
NFT = FC // P
NDT = D // P
NST = T // P
EPS = 1e-05

SWQ = 512.0
SWK = 64.0
SWV = 64.0
SWO = 32.0
SA = 16.0
SV = 4.0


def _build():
    nc = bacc.Bacc("TRN2", target_bir_lowering=False, num_devices=N_CORES)

    x8d = nc.declare_dram_parameter("x8d", [D, M], F8, isOutput=False)
    xh8 = nc.declare_dram_parameter("xh8", [D, M], F16, isOutput=False)  # x/8
    css = nc.declare_dram_parameter("css", [2, P, M], F16, isOutput=False)
    wq8 = nc.declare_dram_parameter("wq8", [NH, P, NKP, P], F8, isOutput=False)
    wk8 = nc.declare_dram_parameter("wk8", [NH, P, NKP, P], F8, isOutput=False)
    wv8 = nc.declare_dram_parameter("wv8", [P, NKP, QC], F8, isOutput=False)
    wo8 = nc.declare_dram_parameter("wo8", [P, NH, NDT, P], F8, isOutput=False)
    wf_t = nc.declare_dram_parameter("wf_t", [NFT, P, NKP, P], F16, isOutput=False)
    wu_t = nc.declare_dram_parameter("wu_t", [NFT, P, NKP, P], F16, isOutput=False)
    wout_t = nc.declare_dram_parameter("wout_t", [NDT, P, NFT, P], F16, isOutput=False)
    y = nc.declare_dram_parameter("y", [D, M], F32, isOutput=True)

    with tile.TileContext(nc) as tc:
        _emit(nc, tc, x8d, xh8, css, wq8, wk8, wv8, wo8, wf_t, wu_t, wout_t, y)
    nc.compile()
    return nc


def _emit(nc, tc, x8d, xh8, css, wq8, wk8, wv8, wo8, wf_t, wu_t, wout_t, y):
    top = ExitStack()
    with top:
        dram_pool = top.enter_context(tc.tile_pool(name="dram", bufs=1, space="DRAM"))
        const = top.enter_context(tc.tile_pool(name="const", bufs=1))

        cc_in = [dram_pool.tile([D, T], F16, name=f"cc_in_{b}") for b in range(B)]
        cc_out = [
            dram_pool.tile([D, T], F16, addr_space="Shared", name=f"cc_out_{b}")
            for b in range(B)
        ]

        ones8 = const.tile([P, 2, P], F8)
        nc.vector.memset(ones8[:], 1.0)
        onescol = const.tile([P, 1], F16)
        nc.vector.memset(onescol[:], SV / (SWV * P))
        eps_sb = const.tile([P, 1], F32)
        nc.vector.memset(eps_sb[:], EPS)
        lnav = const.tile([P, 1], F32)
        nc.vector.memset(lnav[:], float(np.log(SA / SV)))
        bc2 = [const.tile([P, T], F16, name=f"bc2_{b}") for b in range(B)]

        # shared stats pools: one ms psum tile (rotated over the 4 norm
        # passes) + a small ring of fp8 square-pair tiles
        stsb = top.enter_context(tc.tile_pool(name="stsb", bufs=1))
        stps = top.enter_context(tc.tile_pool(name="stps", bufs=1, space="PSUM"))

        # xm pool: one tag-rotated buffer; xm[1]'s DMA WAR-waits on the
        # last ff/up read of xm[0].
        xmp = top.enter_context(tc.tile_pool(name="xmp", bufs=1))

        # ---- first-half state; batch-alternating tiles share tags ----
        half1 = ExitStack()
        ep = half1.enter_context(tc.tile_pool(name="half1", bufs=1))
        cc_sb = [ep.tile([P, T], F16, tag="ccs", name=f"ccs_{b}") for b in range(B)]
        ss_sb = [ep.tile([P, T], F16, tag="sss", name=f"sss_{b}") for b in range(B)]

        def emit_css_dma(b):
            bs = slice(b * T, (b + 1) * T)
            nc.sync.dma_start(out=cc_sb[b][:], in_=css[0][:, bs])
            nc.sync.dma_start(out=ss_sb[b][:], in_=css[1][:, bs])

        bc1 = [ep.tile([P, T], F16, tag="bc1", name=f"bc1_{b}") for b in range(B)]
        ccrs = [ep.tile([P, T], F16, tag="ccrs", name=f"ccrs_{b}") for b in range(B)]
        ssrs = [ep.tile([P, T], F16, tag="ssrs", name=f"ssrs_{b}") for b in range(B)]
        pt_sb = [ep.tile([P, NST], F32, tag="pt", name=f"pt_{b}") for b in range(B)]
        x8t = [ep.tile([P, NKP, T], F8, tag="x8", name=f"x8_{b}") for b in range(B)]

        def w_qk_dma(b, which):
            wsrc = wq8 if which == "q" else wk8
            out = []
            for m in range(NH):
                t = ep.tile([P, NKP, P], F8, tag="wqk", bufs=3,
                            name=f"w{which}_{b}_{m}")
                nc.sync.dma_start(out=t[:], in_=wsrc[m])
                out.append(t)
            return out

        def w_v_dma(b):
            t = ep.tile([P, NKP, QC], F8, tag="wv", bufs=1, name=f"wv_{b}")
            nc.sync.dma_start(out=t[:], in_=wv8[:])
            return t

        def w_o_dma(b):
            t = ep.tile([P, NH, NDT, P], F8, tag="wo", bufs=1, name=f"wo_{b}")
            nc.sync.dma_start(out=t[:], in_=wo8[:])
            return t

        def emit_x8_dma(b):
            bs = slice(b * T, (b + 1) * T)
            for kp in range(NKP):
                nc.sync.dma_start(
                    out=x8t[b][:, kp, :], in_=x8d[kp * P : (kp + 1) * P, bs]
                )

        # ---------------- stats helpers ----------------
        def new_ms(tag):
            return stps.tile([P, T], F32, tag="ms", name=f"ms_{tag}")

        def stats_sq_inline(src3d, ms_ps, tag, eng="mixed"):
            # squares + reduction matmuls emitted together (norm1 passes)
            for pi in range(NPR):
                sq = stsb.tile([P, 2, T], F8, tag="sq", bufs=3,
                               name=f"sq_{tag}_{pi}")
                for j in (0, 1):
                    kp = 2 * pi + j
                    if eng == "dve" or kp % 2 == 1:
                        nc.vector.tensor_mul(
                            sq[:, j, :], src3d[:, kp, :], src3d[:, kp, :]
                        )
                    else:
                        nc.scalar.activation(
                            sq[:, j, :], src3d[:, kp, :], AF.Square
                        )
                for ch in range(T // 512):
                    cs = slice(ch * 512, (ch + 1) * 512)
                    nc.tensor.matmul(
                        ms_ps[:, cs],
                        ones8[:],
                        sq[:, :, cs],
                        start=(pi == 0),
                        stop=(pi == NPR - 1),
                        perf_mode=DR,
                    )

        def stats_sq_anchored(src3d, tag, anchor):
            # DVE squares with an artificial dependency on `anchor` (a
            # [P,1] slice of a tile written by the op the squares must
            # not be scheduled ahead of); reduction mms come later.
            sqs = []
            for pi in range(NPR):
                sq = stsb.tile([P, 2, T], F8, tag="sq", bufs=3,
                               name=f"sq_{tag}_{pi}")
                for j in (0, 1):
                    kp = 2 * pi + j
                    nc.vector.scalar_tensor_tensor(
                        sq[:, j, :], src3d[:, kp, :], anchor,
                        src3d[:, kp, :], ALU.bypass, ALU.mult,
                    )
                sqs.append(sq)
            return sqs

        def stats_mms(sqs, ms_ps, rng):
            for pi in rng:
                for ch in range(T // 512):
                    cs = slice(ch * 512, (ch + 1) * 512)
                    nc.tensor.matmul(
                        ms_ps[:, cs],
                        ones8[:],
                        sqs[pi][:, :, cs],
                        start=(pi == 0),
                        stop=(pi == NPR - 1),
                        perf_mode=DR,
                    )

        def stats_finish(ms_ps, bcast_out, tag):
            lnt = stsb.tile([P, T], F16, tag="lnt", name=f"lnt_{tag}")
            nc.scalar.activation(
                lnt[:], ms_ps[:], AF.Ln, bias=eps_sb[:], scale=1.0 / D
            )
            nc.scalar.activation(bcast_out[:], lnt[:], AF.Exp, scale=-0.5)

        def emit_pt(b, ms_ps):
            # pt[p, st] = rs1[st*P+p] * SV/SWV via 1-col ones matmuls
            for st in range(NST):
                nc.tensor.matmul(
                    ms_ps[:, st : st + 1],
                    bc1[b][:, st * P : (st + 1) * P],
                    onescol[:],
                    start=True,
                    stop=True,
                )
            nc.scalar.activation(pt_sb[b][:], ms_ps[:, 0:NST], AF.Copy)

        def emit_ccss(b):
            nc.vector.tensor_mul(ccrs[b][:], cc_sb[b][:], bc1[b][:])
            nc.vector.tensor_mul(ssrs[b][:], ss_sb[b][:], bc1[b][:])

        # ---------------- phase helpers ----------------
        def emit_qk(b, sp, ap, qpp, qf, kf, insert_pt, ms_ps, wqt, wkt):
            for which, wts, s_w, dst in (
                ("q", wqt, 1.0 / SWQ, qf),
                ("k", wkt, 1.0 / SWK, kf),
            ):
                for m in range(NH):
                    wt = wts[m]
                    ps = qpp.tile([P, T], F32, tag="qk", bufs=2,
                                  name=f"ps{which}_{b}_{m}")
                    for pi in range(NPR):
                        for ch in range(T // 512):
                            cs = slice(ch * 512, (ch + 1) * 512)
                            nc.tensor.matmul(
                                ps[:, cs],
                                wt[:, 2 * pi : 2 * pi + 2, :],
                                x8t[b][:, 2 * pi : 2 * pi + 2, cs],
                                start=(pi == 0),
                                stop=(pi == NPR - 1),
                                perf_mode=DR,
                            )
                    if insert_pt and which == "q" and m == 0:
                        emit_pt(b, ms_ps)
                    main = sp.tile([P, T], F16, tag="rmain", bufs=2,
                                   name=f"rm_{which}_{b}_{m}")
                    nc.vector.scalar_tensor_tensor(
                        main[:], ps[:], s_w, ccrs[b][:], ALU.mult, ALU.mult
                    )
                    rot = sp.tile([P, T], F16, tag="rrot", bufs=2,
                                  name=f"rr_{which}_{b}_{m}")
                    nc.vector.scalar_tensor_tensor(
                        rot[:HALF], ps[HALF:], -s_w, ssrs[b][:HALF],
                        ALU.mult, ALU.mult,
                    )
                    nc.vector.scalar_tensor_tensor(
                        rot[HALF:], ps[:HALF], s_w, ssrs[b][HALF:],
                        ALU.mult, ALU.mult,
                    )
                    out = ap.tile([P, T], F16, tag=f"{which}f{m}",
                                  name=f"{which}f_{b}_{m}")
                    nc.vector.tensor_add(out[:], main[:], rot[:])
                    dst[m] = out

        def emit_v(b, vpp, v8_sb, wv_sb):
            for st in range(NST):
                ps = vpp.tile([P, QC], F32, tag="vps", bufs=2,
                              name=f"psv_{b}_{st}")
                for pi in range(NPR):
                    nc.tensor.matmul(
                        ps[:],
                        x8t[b][:, 2 * pi : 2 * pi + 2, st * P : (st + 1) * P],
                        wv_sb[:, 2 * pi : 2 * pi + 2, :],
                        start=(pi == 0),
                        stop=(pi == NPR - 1),
                        perf_mode=DR,
                    )
                nc.scalar.activation(
                    v8_sb[:, st, :], ps[:], AF.Copy,
                    scale=pt_sb[b][:, st : st + 1],
                )

        def emit_attn_head(b, h, ap_, app, qf, kf, v8_sb, af8b):
            den_ps = app.tile([P, T], F32, tag="den", name=f"den_{b}_{h}")
            at_ps = app.tile([P, T], F32, tag="at", name=f"at_{b}_{h}")
            prs = [None] * (NST // 2)

            def emit_lg(u):
                pr8 = ap_.tile([P, 2, T], F8, tag="pr8", bufs=3,
                               name=f"pr_{b}_{h}_{u}")
                for j in (0, 1):
                    st = 2 * u + j
                    for ch in range(T // 512):
                        cs = slice(ch * 512, (ch + 1) * 512)
                        lg_ps = app.tile([P, 512], F32, tag="lg", bufs=2,
                                         name=f"lg_{b}_{h}_{st}_{ch}")
                        nc.tensor.matmul(
                            lg_ps[:],
                            kf[h][:, st * P : (st + 1) * P],
                            qf[h][:, cs],
                            start=True,
                            stop=True,
                        )
                        nc.scalar.activation(pr8[:, j, cs], lg_ps[:], AF.Exp)
                prs[u] = pr8

            emit_lg(0)
            for u in range(NST // 2):
                if u + 1 < NST // 2:
                    emit_lg(u + 1)
                pr8 = prs[u]
                for ch in range(T // 512):
                    cs = slice(ch * 512, (ch + 1) * 512)
                    nc.tensor.matmul(
                        den_ps[:, cs],
                        ones8[:],
                        pr8[:, :, cs],
                        start=(u == 0),
                        stop=(u == NST // 2 - 1),
                        perf_mode=DR,
                    )
                for ch in range(T // 512):
                    cs = slice(ch * 512, (ch + 1) * 512)
                    nc.tensor.matmul(
                        at_ps[:, cs],
                        v8_sb[:, 2 * u : 2 * u + 2, h * H : (h + 1) * H],
                        pr8[:, :, cs],
                        start=(u == 0),
                        stop=(u == NST // 2 - 1),
                        perf_mode=DR,
                    )
            # af = at * (SA/SV) / den  via  exp(ln(SA/SV) - ln(den))
            lnd = ap_.tile([P, T], F16, tag="lnd", bufs=2, name=f"lnd_{b}_{h}")
            nc.scalar.activation(lnd[:], den_ps[:], AF.Ln)
            rs = ap_.tile([P, T], F16, tag="rsd", bufs=2, name=f"rsd_{b}_{h}")
            nc.scalar.activation(
                rs[:], lnd[:], AF.Exp, scale=-1.0, bias=lnav[:]
            )
            nc.vector.tensor_mul(af8b[:, h, :], at_ps[:], rs[:])

        def emit_oproj(b, sp, opp, af8b, wo_sb):
            bs = slice(b * T, (b + 1) * T)

            def xh_dma(dt):
                t = sp.tile([P, T], F16, tag="xh", bufs=3, name=f"xh_{b}_{dt}")
                nc.sync.dma_start(
                    out=t[:], in_=xh8[dt * P : (dt + 1) * P, bs]
                )
                return t
            xhl = [xh_dma(dt) for dt in range(2)] + [None] * (NDT - 2)
            for dt in range(NDT):
                if dt + 2 < NDT:
                    xhl[dt + 2] = xh_dma(dt + 2)
                ps = opp.tile([P, T], F32, tag="ops", bufs=3,
                              name=f"pso_{b}_{dt}")
                for u in range(NH // 2):
                    for ch in range(T // 512):
                        cs = slice(ch * 512, (ch + 1) * 512)
                        nc.tensor.matmul(
                            ps[:, cs],
                            wo_sb[:, 2 * u : 2 * u + 2, dt, :],
                            af8b[:, 2 * u : 2 * u + 2, cs],
                            start=(u == 0),
                            stop=(u == NH // 2 - 1),
                            perf_mode=DR,
                        )
                osb = sp.tile([P, T], F16, tag="osb", bufs=2,
                              name=f"osb_{b}_{dt}")
                nc.vector.scalar_tensor_tensor(
                    osb[:], ps[:], 1.0 / (SA * SWO), xhl[dt][:],
                    ALU.mult, ALU.add,
                )
                nc.sync.dma_start(
                    out=cc_in[b][dt * P : (dt + 1) * P, :], in_=osb[:]
                )
            nc.gpsimd.collective_compute(
                "AllReduce",
                ALU.add,
                replica_groups=[list(range(N_CORES))],
                ins=[cc_in[b][:, :]],
                outs=[cc_out[b][:, :]],
            )

        xm = [None, None]

        def emit_xm_dma(b):
            xm[b] = xmp.tile([P, NKP, T], F16, tag="xm", bufs=1, name=f"xm_{b}")
            for kp in range(NKP):
                nc.gpsimd.dma_start(
                    out=xm[b][:, kp, :],
                    in_=cc_out[b][kp * P : (kp + 1) * P, :],
                )

        def emit_mlp(b, sp, hp, stat_hook=None):
            # When stat_hook is given, bc2[b] is only produced inside the
            # hook at (m=1, 'f'); evictions of earlier blocks must be
            # deferred until then (they read bc2).  PSUM bufs=3 holds the
            # deferred accumulators.
            hsb = []
            ffs_tiles = {}

            def evict(m, which, ps):
                nt = sp.tile([P, T], F16, tag=f"nt_{which}", bufs=2,
                             name=f"nt{which}_{b}_{m}")
                nc.vector.scalar_tensor_tensor(
                    nt[:], ps[:], 1.0, bc2[b][:], ALU.mult, ALU.mult
                )
                if which == "f":
                    ffs = sp.tile([P, T], F16, tag="ffs", bufs=2,
                                  name=f"ff_{b}_{m}")
                    nc.scalar.activation(ffs[:], nt[:], AF.Silu)
                    ffs_tiles[m] = ffs
                else:
                    ht = hp.tile([P, T], F16, tag=f"h{m}", name=f"h_{b}_{m}")
                    nc.vector.tensor_mul(ht[:], nt[:], ffs_tiles[m][:])
                    hsb.append(ht)

            bc_ready = stat_hook is None
            deferred = []
            with ExitStack() as psc:
                pp = psc.enter_context(
                    tc.tile_pool(name=f"mlpp{b}", bufs=1, space="PSUM")
                )
                for m in range(NFT):
                    for which, wsrc in (("f", wf_t), ("u", wu_t)):
                        wt = sp.tile([P, NKP, P], F16, tag="wffu", bufs=2,
                                     name=f"w{which}_{b}_{m}")
                        nc.sync.dma_start(out=wt[:], in_=wsrc[m])
                        ps = pp.tile([P, T], F32, tag="psfu", bufs=3,
                                     name=f"ps{which}_{b}_{m}")
                        for kp in range(NKP):
                            for ch in range(T // 512):
                                cs = slice(ch * 512, (ch + 1) * 512)
                                nc.tensor.matmul(
                                    ps[:, cs],
                                    wt[:, kp, :],
                                    xm[b][:, kp, cs],
                                    start=(kp == 0),
                                    stop=(kp == NKP - 1),
                                )
                        if stat_hook is not None:
                            if stat_hook(m, which):
                                bc_ready = True
                                for dm, dw, dps in deferred:
                                    evict(dm, dw, dps)
                                deferred = []
                        if bc_ready:
                            evict(m, which, ps)
                        else:
                            deferred.append((m, which, ps))
            return hsb

        def emit_wout(b, sp, pp, hsb):
            bs = slice(b * T, (b + 1) * T)
            for dt in range(NDT):
                wt = sp.tile([P, NFT, P], F16, tag="wot", bufs=3,
                             name=f"wot_{b}_{dt}")
                nc.sync.dma_start(out=wt[:], in_=wout_t[dt])
                ps = pp.tile([P, T], F32, tag="pso2", bufs=2,
                             name=f"pso2_{b}_{dt}")
                for m in range(NFT):
                    for ch in range(T // 512):
                        cs = slice(ch * 512, (ch + 1) * 512)
                        nc.tensor.matmul(
                            ps[:, cs],
                            wt[:, m, :],
                            hsb[m][:, cs],
                            start=(m == 0),
                            stop=(m == NFT - 1),
                        )
                xr = sp.tile([P, T], F16, tag="xr", bufs=3,
                             name=f"xr_{b}_{dt}")
                nc.sync.dma_start(
                    out=xr[:],
                    in_=cc_out[b][dt * P : (dt + 1) * P, :],
                )
                ysb = sp.tile([P, T], F32, tag="ysb", bufs=3,
                              name=f"ysb_{b}_{dt}")
                nc.vector.scalar_tensor_tensor(
                    ysb[:], xr[:], 0.125, ps[:], ALU.mult, ALU.add
                )
                nc.sync.dma_start(out=y[dt * P : (dt + 1) * P, bs], in_=ysb[:])

        def make_stat_hook(sqs, ms_ps, bc_out, tag):
            def hook(m, which):
                if m == 0 and which == "f":
                    stats_mms(sqs, ms_ps, range(0, 6))
                elif m == 0 and which == "u":
                    stats_mms(sqs, ms_ps, range(6, 12))
                elif m == 1 and which == "f":
                    stats_mms(sqs, ms_ps, range(12, NPR))
                    stats_finish(ms_ps, bc_out, tag)
                    return True
                return False
            return hook

        # ================= emission schedule =================
        emit_x8_dma(0)
        emit_css_dma(0)
        wqt0 = w_qk_dma(0, "q")
        wkt0 = w_qk_dma(0, "k")
        wvt0 = w_v_dma(0)

        qf = [[None] * NH for _ in range(B)]
        kf = [[None] * NH for _ in range(B)]
        v8_sb = [ep.tile([P, NST, QC], F8, tag="v8", name=f"v8_{b}")
                 for b in range(B)]
        af8 = [ep.tile([P, NH, T], F8, tag="af8", name=f"af8_{b}")
               for b in range(B)]

        # ---- b0 projections ----
        with ExitStack() as ph:
            sp = ph.enter_context(tc.tile_pool(name="prj0", bufs=1))
            qpp = ph.enter_context(tc.tile_pool(name="qk0", bufs=1, space="PSUM"))
            vpp = ph.enter_context(tc.tile_pool(name="v0", bufs=1, space="PSUM"))
            ms_n1b0 = new_ms("n1b0")
            stats_sq_inline(x8t[0], ms_n1b0, "n1b0")
            stats_finish(ms_n1b0, bc1[0], "n1b0")
            emit_ccss(0)
            emit_qk(0, sp, ep, qpp, qf[0], kf[0], True, ms_n1b0, wqt0, wkt0)
            emit_v(0, vpp, v8_sb[0], wvt0)

        # ---- b0 attention (+ b1 x8 / wq / wo prefetch) ----
        with ExitStack() as ah:
            ap_ = ah.enter_context(tc.tile_pool(name="attb0", bufs=1))
            app = ah.enter_context(tc.tile_pool(name="attp0", bufs=1, space="PSUM"))
            emit_attn_head(0, 0, ap_, app, qf[0], kf[0], v8_sb[0], af8[0])
            emit_x8_dma(1)
            emit_css_dma(1)
            emit_attn_head(0, 1, ap_, app, qf[0], kf[0], v8_sb[0], af8[0])
            wqt1 = w_qk_dma(1, "q")
            wo0 = w_o_dma(0)
            for h in range(2, NH):
                emit_attn_head(0, h, ap_, app, qf[0], kf[0], v8_sb[0], af8[0])

        # ---- b1 norm1 stats (DVE squares) ----
        ms_n1b1 = new_ms("n1b1")
        stats_sq_inline(x8t[1], ms_n1b1, "n1b1", eng="dve")
        stats_finish(ms_n1b1, bc1[1], "n1b1")
        emit_ccss(1)
        emit_pt(1, ms_n1b1)

        # ---- b1 q/k, then b0 o-proj + AR0, then b1 v ----
        with ExitStack() as ph:
            sp = ph.enter_context(tc.tile_pool(name="prj1", bufs=1))
            wkt1 = w_qk_dma(1, "k")
            wvt1 = w_v_dma(1)
            with ExitStack() as qh:
                qpp = qh.enter_context(
                    tc.tile_pool(name="qk1", bufs=1, space="PSUM")
                )
                emit_qk(1, sp, ep, qpp, qf[1], kf[1], False, None, wqt1, wkt1)

            with ExitStack() as oh:
                osp = oh.enter_context(tc.tile_pool(name="op0", bufs=1))
                opp = oh.enter_context(
                    tc.tile_pool(name="opp0", bufs=1, space="PSUM")
                )
                emit_oproj(0, osp, opp, af8[0], wo0)

            with ExitStack() as vh:
                vpp = vh.enter_context(tc.tile_pool(name="v1", bufs=1, space="PSUM"))
                emit_v(1, vpp, v8_sb[1], wvt1)

        # ---- b1 attention (+ b0 xm prefetch) ----
        with ExitStack() as ah:
            ap_ = ah.enter_context(tc.tile_pool(name="attb1", bufs=1))
            app = ah.enter_context(tc.tile_pool(name="attp1", bufs=1, space="PSUM"))
            emit_attn_head(1, 0, ap_, app, qf[1], kf[1], v8_sb[1], af8[1])
            wo1 = w_o_dma(1)
            emit_xm_dma(0)
            for h in range(1, NH):
                emit_attn_head(1, h, ap_, app, qf[1], kf[1], v8_sb[1], af8[1])

        # ---- b1 o-proj + AR1 ----
        with ExitStack() as oh:
            osp = oh.enter_context(tc.tile_pool(name="op1", bufs=1))
            opp = oh.enter_context(tc.tile_pool(name="opp1", bufs=1, space="PSUM"))
            emit_oproj(1, osp, opp, af8[1], wo1)

        # ---- b0 norm2 squares: anchored behind b1's last attention
        # eviction so they fill DVE gaps during b1's o-proj ----
        sq0 = stats_sq_anchored(xm[0], "n2b0", af8[1][:, NH - 1, 0:1])
        ms_n2b0 = new_ms("n2b0")

        half1.close()

        # ---- b0 MLP (stats mms for b0 norm2 interleaved via hook) ----
        with ExitStack() as mh:
            sp = mh.enter_context(tc.tile_pool(name="mlp0", bufs=1))
            hp = mh.enter_context(tc.tile_pool(name="h0", bufs=1))
            hsb = emit_mlp(
                0, sp, hp,
                stat_hook=make_stat_hook(sq0, ms_n2b0, bc2[0], "n2b0"),
            )
            emit_xm_dma(1)
            with ExitStack() as wh:
                wsp = wh.enter_context(tc.tile_pool(name="wo2_0", bufs=1))
                wpp = wh.enter_context(
                    tc.tile_pool(name="wo2p0", bufs=1, space="PSUM")
                )
                emit_wout(0, wsp, wpp, hsb)

            # b1 norm2 squares: anchored behind b0's last MLP h-mul;
            # they fill DVE gaps during b0's wout phase
            sq1 = stats_sq_anchored(xm[1], "n2b1", hsb[NFT - 1][:, 0:1])
            ms_n2b1 = new_ms("n2b1")

        # ---- b1 MLP (+ b1 norm2 stats mms via hook) ----
        with ExitStack() as mh1:
            sp = mh1.enter_context(tc.tile_pool(name="mlp1", bufs=1))
            hp = mh1.enter_context(tc.tile_pool(name="h1", bufs=1))
            hsb = emit_mlp(
                1, sp, hp,
                stat_hook=make_stat_hook(sq1, ms_n2b1, bc2[1], "n2b1"),
            )
            with ExitStack() as wh:
                wsp = wh.enter_context(tc.tile_pool(name="wo2_1", bufs=1))
                wpp = wh.enter_context(
                    tc.tile_pool(name="wo2p1", bufs=1, space="PSUM")
                )
                emit_wout(1, wsp, wpp, hsb)


_NC_CACHE = {}


def _get_nc():
    if "nc" not in _NC_CACHE:
        _NC_CACHE["nc"] = _build()
    return _NC_CACHE["nc"]


def _host_prep(x, sin, cos, attn_norm_w, ff_norm_w, wq, wk, wv, wo, w_ff, w_up, w_out):
    f16 = np.float16
    x2 = np.asarray(x, np.float32).reshape(M, D)
    xT = np.ascontiguousarray(x2.T)

    sinT = np.asarray(sin, np.float32).reshape(M, HALF).T
    cosT = np.asarray(cos, np.float32).reshape(M, HALF).T
    cc = np.concatenate([cosT, cosT], axis=0)
    ss = np.concatenate([sinT, sinT], axis=0)
    css = np.stack([cc, ss]).astype(f16)

    anw = np.asarray(attn_norm_w, np.float32)[:, None]
    fnw = np.asarray(ff_norm_w, np.float32)[:, None]
    wqn = (anw * np.asarray(wq, np.float32)) * (H ** -0.5) * SWQ
    wkn = anw * np.asarray(wk, np.float32) * SWK
    wvn = anw * np.asarray(wv, np.float32) * SWV
    won = np.asarray(wo, np.float32) * SWO
    wfn = fnw * np.asarray(w_ff, np.float32)
    wun = fnw * np.asarray(w_up, np.float32)
    w_out = np.asarray(w_out, np.float32)

    def mtile(w):
        # [K, F] -> [F/P, P, K/P, P] with [m, p, kp, j] = w[kp*P+p, m*P+j]
        K, F = w.shape
        return np.ascontiguousarray(
            w.reshape(K // P, P, F // P, P).transpose(2, 1, 0, 3)
        )

    x8_full = xT.astype(E4)
    xh8_full = (xT * 0.125).astype(f16)

    in_maps = []
    for c in range(N_CORES):
        qs = slice(c * QC, (c + 1) * QC)
        fs = slice(c * FC, (c + 1) * FC)
        in_maps.append(
            {
                "x8d": x8_full,
                "xh8": xh8_full,
                "css": css,
                "wq8": mtile(wqn[:, qs]).astype(E4),
                "wk8": mtile(wkn[:, qs]).astype(E4),
                # [p, kp, f] = wvn[kp*P+p, f]
                "wv8": np.ascontiguousarray(
                    wvn[:, qs].reshape(NKP, P, QC).transpose(1, 0, 2)
                ).astype(E4),
                # [p, h, dt, j] = wo[c*QC + h*P + p, dt*P + j]
                "wo8": np.ascontiguousarray(
                    won[qs, :].reshape(NH, P, NDT, P).transpose(1, 0, 2, 3)
                ).astype(E4),
                "wf_t": mtile(wfn[:, fs]).astype(f16),
                "wu_t": mtile(wun[:, fs]).astype(f16),
                "wout_t": mtile(w_out[fs, :]).astype(f16),
            }
        )
    return in_maps


def kernel(**inputs) -> np.ndarray:
    nc = _get_nc()
    in_maps = _host_prep(**inputs)
    res = run_bass_kernel_spmd(
        nc, in_maps, core_ids=list(range(N_CORES)), trace=False
    )
    acc = res.results[0]["y"].astype(np.float64)
    for c in range(1, N_CORES):
        acc += res.results[c]["y"]
    return np.ascontiguousarray(acc.T).astype(np.float32).reshape(B, T, D)
